# revision 1
# baseline (speedup 1.0000x reference)
"""Multi-head causal attention (B=2, S=2048, E=1024, H=16, D=64) on 8 TRN2
NeuronCores.

Sharding: 4-way tensor-parallel over heads x 2-way data-parallel over batch.
Core c handles batch b = c//4 and head group g = c%4 (heads 4g..4g+3).

Per-core device program (all matmuls bf16, fp32 PSUM accumulate):
  - QT/KT = (X @ Wq/Wk)^T computed directly in [d, s] layout
    (lhsT = W slice, rhs = X^T), V in [s, d] layout (lhsT = X^T, rhs = Wv),
    with a ones-column appended to V per head (V_aug) so the softmax
    denominators fall out of the PV matmul for free.
  - Scores computed TRANSPOSED: ST[k, q] = (Q K^T)^T via lhsT=KT, rhs=QT.
    The two heads of a pair run on PE row groups 0-63 / 64-127 concurrently
    (K=64 contraction uses disjoint row strips of the 128x128 array).
    Causal mask added with an extra accumulating matmul (identity x
    precomputed -1e9 triangular tile) on diagonal blocks; fully-masked
    blocks are skipped.
  - P = exp(ST/8) on ScalarE (PSUM -> SBUF, bf16 out). No running max:
    scores are O(30) so exp is safe in fp32.
  - ctxT[d, q] (+ sums row) = V_aug^T @ P^T via lhsT=V_aug, rhs=P,
    accumulated over k tiles in PSUM.
  - Normalize: linv = 1/sums (DVE), broadcast across partitions via a
    DRAM-bounce DMA, multiply on DVE -> ctxn (bf16).
  - outT partial [e, q] = Wo_shard^T @ ctxn per q chunk, interleaved with
    the attention of later chunks, DMA'd out as fp32.

Host side: gather 8 partial outT tensors, out[b] = sum_g partial^T + bo.
"""

import numpy as np
import ml_dtypes

import bass_rust
import concourse.bass as bass
import concourse.tile as tile
from concourse import mybir
from concourse.tile import TileContext
from concourse.bass_utils import run_bass_kernel_spmd
from concourse.vector_clock import ScopedClock

# ---------------------------------------------------------------------------
# Workaround: this environment's walrus build caps the number of sync-wait
# commands encodable on a single instruction. Redistribute excess waits onto
# single-wait NOPs inserted immediately before the instruction on the same
# engine queue (program order preserves the blocking semantics).
# ---------------------------------------------------------------------------


def _patched_drain_and_barrier(self, tick_clock, wait_clock):
    nop0 = self.nc.sync.nop(nofuse=True)
    wait_clock.add_sem_waits(nop0.ins, ScopedClock({None: tick_clock.global_clock}))
    si = nop0.ins.sync_info
    if si is not None and si.on_wait and len(si.on_wait) > 1:
        waits = list(si.on_wait)
        nop0.ins.sync_info = bass_rust.SyncInfo(
            on_wait=[waits[0]], on_update=list(si.on_update or [])
        )
        for w in waits[1:]:
            n = self.nc.sync.nop(nofuse=True)
            n.ins.sync_info = bass_rust.SyncInfo(on_wait=[w], on_update=[])
    self.nc.sync.drain()
    self.nc.all_engine_barrier()
    assert self.sems is not None
    popped = self.nc._tile_sem_poison_stack.pop()
    assert popped is self._sem_poison
    self.nc.clear_and_free_semaphores(list(self.sems.allocated().values()))
    self.nc.all_engine_barrier()


tile.TileContext._drain_and_barrier = _patched_drain_and_barrier

MAX_WAITS = 1


def split_excess_waits(nc, max_waits=None):
    if max_waits is None:
        max_waits = MAX_WAITS
    for f in nc.m.functions:
        for bb in f.blocks:
            insts = bb.instructions
            out = []
            changed = False
            for inst in insts:
                si = inst.sync_info
                waits = list(si.on_wait) if si is not None and si.on_wait else []
                if len(waits) > max_waits:
                    changed = True
                    excess, keep = waits[:-max_waits], waits[-max_waits:]
                    for w in excess:
                        out.append(mybir.InstNoOp(
                            name=nc.get_next_instruction_name(),
                            engine=inst.engine,
                            bass_nofuse=True,
                            sync_info=mybir.SyncInfo(on_wait=[w], on_update=[]),
                        ))
                    inst.sync_info = mybir.SyncInfo(
                        on_wait=keep, on_update=list(si.on_update or [])
                    )
                out.append(inst)
            if changed:
                bb.instructions = out


# ---------------------------------------------------------------------------
# Problem constants (hardcoded per the harness contract).
# ---------------------------------------------------------------------------

B, S, E = 2, 2048, 1024
H, D = 16, 64
NCORES, TP = 8, 4
HPC = H // TP          # heads per core = 4
DH = HPC * D           # head-dim columns per core = 256
P = 128                # partitions
QC = 512               # q chunk (free dim of score matmuls)
NQC = S // QC          # 4 q chunks
NKT = S // P           # 16 k tiles
SCALE = 1.0 / np.sqrt(np.float32(D))
NEG = -1.0e9

# Tunable scheduling knobs (sweepable via OPTS; defaults = current best)
OPTS = {
    "pj_bufs": 4,
    "st_bufs": 5,
    "stx_bufs": 6,
    "po_bufs": 1,
    "narrow": True,
}

BF = mybir.dt.bfloat16
F32 = mybir.dt.float32
Exp = mybir.ActivationFunctionType.Exp
MULT = mybir.AluOpType.mult


class _Persist:
    pass


def _declare_io(nc):
    io = _Persist()
    # inputs are pre-arranged on host to [partition, chunk, free] so every
    # DMA is contiguous per partition (strided 512B-chunk descriptors made
    # the input phase descriptor-bound otherwise)
    io.xt = nc.dram_tensor("xt", [P, E // P, S], BF, kind="ExternalInput").ap()
    io.wq = nc.dram_tensor("wq", [P, E // P, DH], BF, kind="ExternalInput").ap()
    io.wk = nc.dram_tensor("wk", [P, E // P, DH], BF, kind="ExternalInput").ap()
    io.wv = nc.dram_tensor("wv", [P, E // P, DH], BF, kind="ExternalInput").ap()
    io.wo = nc.dram_tensor("wo", [P, DH // P, E], BF, kind="ExternalInput").ap()
    io.outp = nc.dram_tensor("outp", [E, S], F32, kind="ExternalOutput").ap()
    return io


ET = E // P   # 8 e tiles
NM = QC // P  # 4 k-tiles per q chunk


def _persistent(ctx, tc):
    nc = tc.nc
    ps = _Persist()
    consts = ctx.enter_context(tc.tile_pool(name="consts", bufs=1))
    ps.dram = ctx.enter_context(tc.tile_pool(name="dram", bufs=2, space="DRAM"))

    # xt split per e-tile so projections start as soon as each chunk lands;
    # kt/v split per k-tile so attention starts before projections finish.
    ps.xt_t = [consts.tile([P, S], BF, tag=f"xt{et}", name=f"xt{et}")
               for et in range(ET)]
    ps.wq_sb = consts.tile([P, ET, DH], BF, tag="wq")
    ps.wk_sb = consts.tile([P, ET, DH], BF, tag="wk")
    ps.wv_sb = consts.tile([P, ET, DH], BF, tag="wv")
    ps.wo_sb = consts.tile([P, DH // P, E], BF, tag="wo")
    ps.qt_q = [consts.tile([P, DH // P, QC], BF, tag=f"qt{qc}",
                           name=f"qt{qc}") for qc in range(NQC)]
    ps.kt_t = [consts.tile([P, DH // P, P], BF, tag=f"kt{ki}", name=f"kt{ki}")
               for ki in range(NKT)]
    ps.v_t = [consts.tile([P, HPC, 66], BF, tag=f"v{ki}", name=f"v{ki}")
              for ki in range(NKT)]
    ps.ctxn_q = [consts.tile([P, DH // P, QC], BF, tag=f"ctxn{qc}",
                             name=f"ctxn{qc}") for qc in range(NQC)]
    ps.ident = consts.tile([P, P], BF, tag="ident")
    ps.masks = consts.tile([P, NM, QC], BF, tag="masks")

    # identity (for the mask-add matmul)
    nc.gpsimd.memset(ps.ident[:], 0.0)
    nc.gpsimd.affine_select(
        out=ps.ident[:], in_=ps.ident[:],
        compare_op=mybir.AluOpType.not_equal, fill=1.0,
        base=0, pattern=[[-1, P]], channel_multiplier=1,
    )
    # mask[m][p, fq] = 0 where fq >= p + 128*m else NEG
    for m in range(NM):
        nc.gpsimd.memset(ps.masks[:, m, :], 0.0)
        nc.gpsimd.affine_select(
            out=ps.masks[:, m, :], in_=ps.masks[:, m, :],
            compare_op=mybir.AluOpType.is_ge, fill=NEG,
            base=-P * m, pattern=[[1, QC]], channel_multiplier=-1,
        )
    # ones column for V_aug
    for ki in range(NKT):
        nc.gpsimd.memset(ps.v_t[ki][:, :, 64:66], 0.0)
        nc.gpsimd.memset(ps.v_t[ki][:, :, 64:65], 1.0)

    ps.stx_pool = ctx.enter_context(tc.tile_pool(name="stx", bufs=OPTS["stx_bufs"]))
    ps.ctxu_pool = ctx.enter_context(tc.tile_pool(name="ctxu", bufs=4))
    ps.linv_pool = ctx.enter_context(tc.tile_pool(name="linv", bufs=8))
    ps.ob_pool = ctx.enter_context(tc.tile_pool(name="ob", bufs=3))
    return ps


def _iteration(tc, io, ps):
    nc = tc.nc
    xt_t, qt_q, kt_t, v_t, ctxn_q = ps.xt_t, ps.qt_q, ps.kt_t, ps.v_t, ps.ctxn_q
    wq_sb, wk_sb, wv_sb, wo_sb = ps.wq_sb, ps.wk_sb, ps.wv_sb, ps.wo_sb
    ident, masks = ps.ident, ps.masks

    nc.sync.dma_start(wq_sb[:], io.wq)
    nc.sync.dma_start(wk_sb[:], io.wk)
    nc.sync.dma_start(wv_sb[:], io.wv)
    nc.sync.dma_start(wo_sb[:], io.wo)
    for et in range(ET):
        nc.sync.dma_start(xt_t[et][:], io.xt[:, et, :])

    # ---- projections (qc-major so attention for early q chunks can start
    # while later projections are still running) ----------------------------
    with tc.tile_pool(name="pj", bufs=OPTS["pj_bufs"], space="PSUM") as pjp:
        for qc in range(NQC):
            # QT / KT for this q chunk: [d, q] layout
            for w_sb, is_q in ((wq_sb, True), (wk_sb, False)):
                for dt in range(DH // P):
                    psum = pjp.tile([P, QC], F32, tag="pj", name="pj")
                    for et in range(ET):
                        nc.tensor.matmul(
                            psum[:],
                            lhsT=w_sb[:, et, dt * P:(dt + 1) * P],
                            rhs=xt_t[et][:, qc * QC:(qc + 1) * QC],
                            start=(et == 0), stop=(et == ET - 1),
                        )
                    if is_q:
                        nc.vector.tensor_copy(
                            out=qt_q[qc][:, dt, :], in_=psum[:]
                        )
                    else:
                        for j in range(NM):
                            nc.vector.tensor_copy(
                                out=kt_t[qc * NM + j][:, dt, :],
                                in_=psum[:, j * P:(j + 1) * P],
                            )
            # V for this chunk's k-tiles: [s, d] layout, per-head 66-wide slots
            for st in range(qc * NM, (qc + 1) * NM):
                psum = pjp.tile([P, DH], F32, tag="pjv", name="pjv")
                for et in range(ET):
                    nc.tensor.matmul(
                        psum[:],
                        lhsT=xt_t[et][:, st * P:(st + 1) * P],
                        rhs=wv_sb[:, et, :],
                        start=(et == 0), stop=(et == ET - 1),
                    )
                nc.vector.tensor_copy(
                    out=v_t[st][:, :, 0:64],
                    in_=psum.rearrange("p (h d) -> p h d", h=HPC),
                )

    # ---- attention + interleaved output projection ------------------------
    # q-chunk outer; head pairs inner. The two heads of a pair use PE row
    # groups 0-63 / 64-127 concurrently, with both score tiles in one 2-bank
    # PSUM tensor so a single wide exp covers them.
    with tc.tile_pool(name="pv", bufs=1, space="PSUM") as pvp, \
         tc.tile_pool(name="st", bufs=OPTS["st_bufs"], space="PSUM") as stp, \
         tc.tile_pool(name="po", bufs=OPTS["po_bufs"], space="PSUM") as pop:
        for qc in range(NQC):
            nk = (qc + 1) * NM  # causal k-tiles for this chunk
            for hp in range(HPC // 2):
                cc = hp
                pvs = [pvp.tile([P, QC], F32, tag=f"pv{i}", name=f"pv{i}")
                       for i in range(2)]
                for ki in range(nk):
                    diag = ki >= qc * NM
                    m = ki - qc * NM if diag else 0
                    off = P * m if (diag and OPTS["narrow"]) else 0
                    # per-head score tiles; QK pair emitted adjacently so the
                    # two matmuls run on disjoint PE row groups concurrently
                    sts = []
                    for i in range(2):
                        pr = 64 * i
                        st_ps = stp.tile([P, QC], F32, tag="st", name="st")
                        nc.tensor.matmul(
                            st_ps[:, off:],
                            lhsT=kt_t[ki][pr:pr + 64, cc, :],
                            rhs=qt_q[qc][pr:pr + 64, cc, off:],
                            start=True, stop=not diag,
                        )
                        sts.append(st_ps)
                    if diag:
                        # only cols [off, off+P) are partially masked
                        for i in range(2):
                            nc.tensor.matmul(
                                sts[i][:, off:off + P],
                                lhsT=ident[:],
                                rhs=masks[:, m, off:off + P],
                                start=False, stop=True,
                            )
                    for i in range(2):
                        h = 2 * hp + i
                        stx = ps.stx_pool.tile([P, QC], BF, tag="stx",
                                               name="stx")
                        nc.scalar.activation(
                            out=stx[:, off:], in_=sts[i][:, off:], func=Exp,
                            scale=float(SCALE),
                        )
                        nc.tensor.matmul(
                            pvs[i][0:65, off:],
                            lhsT=v_t[ki][:, h, 0:65],
                            rhs=stx[:, off:],
                            start=(ki == 0), stop=(ki == nk - 1),
                        )
                # normalize: ctxn[d, q] = ctx[d, q] / sums[q]
                for i in range(2):
                    ctxu = ps.ctxu_pool.tile([65, QC], F32, tag="ctxu",
                                             name="ctxu")
                    nc.vector.tensor_copy(out=ctxu[:], in_=pvs[i][0:65, :])
                    linv = ps.linv_pool.tile([1, QC], F32, tag="linv",
                                             name="linv")
                    nc.vector.reciprocal(linv[:], ctxu[64:65, :])
                    linv_d = ps.dram.tile([1, QC], F32, tag="linvd",
                                          name="linvd")
                    nc.sync.dma_start(linv_d[:], linv[:])
                    linv_bc = ps.linv_pool.tile([64, QC], F32, tag="linvbc",
                                                name="linvbc")
                    nc.sync.dma_start(
                        linv_bc[:], linv_d[0:1, :].to_broadcast((64, QC))
                    )
                    nc.vector.tensor_tensor(
                        ctxn_q[qc][64 * i:64 * i + 64, cc, :],
                        ctxu[0:64, :], linv_bc[:], MULT,
                    )
            # output projection for this q chunk: outT[e, q]. For the last
            # chunk the PV banks are free afterwards — borrow them so the
            # tail out-projection triple-buffers instead of serializing
            # through the single po slot.
            for et in range(ET):
                if qc == NQC - 1 and et % 3:
                    psum = pvp.tile([P, QC], F32, tag=f"pv{et % 2}",
                                    name="povv")
                else:
                    psum = pop.tile([P, QC], F32, tag="po", name="po")
                for cc in range(DH // P):
                    nc.tensor.matmul(
                        psum[:],
                        lhsT=wo_sb[:, cc, et * P:(et + 1) * P],
                        rhs=ctxn_q[qc][:, cc, :],
                        start=(cc == 0), stop=(cc == DH // P - 1),
                    )
                ob = ps.ob_pool.tile([P, QC], F32, tag="ob", name="ob")
                nc.vector.tensor_copy(out=ob[:], in_=psum[:])
                nc.sync.dma_start(
                    io.outp.rearrange("(eo p) q -> p eo q", p=P)[
                        :, et, qc * QC:(qc + 1) * QC
                    ],
                    ob[:],
                )


_NC_CACHE = {}


def build_nc(iters=1):
    if iters not in _NC_CACHE:
        from contextlib import ExitStack
        nc = bass.Bass("TRN2", target_bir_lowering=False, debug=False)
        with TileContext(nc) as tc, ExitStack() as es:
            io = _declare_io(nc)
            ps = _persistent(es, tc)
            for _ in range(iters):
                _iteration(tc, io, ps)
        split_excess_waits(nc)
        _NC_CACHE[iters] = nc
    return _NC_CACHE[iters]


def make_in_maps(embeddings, wq, wk, wv, wo):
    bf = ml_dtypes.bfloat16
    in_maps = []
    for c in range(NCORES):
        b, g = c // TP, c % TP
        cols = slice(g * DH, (g + 1) * DH)
        def _arr(a):  # [(c p), f] -> [p, c, f] contiguous
            c = a.shape[0] // 128
            return np.ascontiguousarray(
                a.reshape(c, 128, a.shape[1]).transpose(1, 0, 2)).astype(bf)
        in_maps.append({
            "xt": _arr(embeddings[b].T),
            "wq": _arr(wq[:, cols]),
            "wk": _arr(wk[:, cols]),
            "wv": _arr(wv[:, cols]),
            "wo": _arr(wo[cols, :]),
        })
    return in_maps


def assemble(results, bo):
    out = np.zeros((B, S, E), dtype=np.float32)
    for c in range(NCORES):
        b = c // TP
        out[b] += results[c]["outp"].T
    out += bo.astype(np.float32)
    return out


def kernel(embeddings, wq, wk, wv, wo, bo):
    embeddings = np.asarray(embeddings)
    nc = build_nc()
    in_maps = make_in_maps(embeddings, np.asarray(wq), np.asarray(wk),
                           np.asarray(wv), np.asarray(wo))
    res = run_bass_kernel_spmd(nc, in_maps, core_ids=list(range(NCORES)),
                               trace=False)
    return assemble(res.results, np.asarray(bo))



# revision 12
# speedup vs baseline: 2.6110x; 2.6110x over previous
"""Multi-head causal attention (B=2, S=2048, E=1024, H=16, D=64) on 8 TRN2
NeuronCores.

Sharding: 4-way tensor-parallel over heads x 2-way data-parallel over batch.
Core c handles batch b = c//4 and head group g = c%4 (heads 4g..4g+3).

Per-core device program (v2):
  - QT/KT = (X @ Wq/Wk)^T computed in [d, s] layout via bf16 matmuls
    (lhsT = W slice, rhs = X^T), V in [s, d] layout, ones column appended
    per head (V_aug) so softmax denominators fall out of the PV matmul.
  - Q/K PSUM results are stored as fp8e4 tiles with an extra zero "slot"
    dimension so the score matmuls can run in MatmulPerfMode.DoubleRow
    (0.5 cycles/row): lhsT = kt8[64, 2, 128] (slot 1 = zeros), rhs =
    qt8[64, 2, q]; out ST[k, q] in PSUM. Scores for the two heads of a
    pair target one [P, 2, QC] two-bank PSUM tile, so a single wide exp
    (ScalarE, PSUM -> SBUF bf16) covers both heads.
  - Causal masking is done post-exp: Pool-engine affine_select zeroes the
    lower triangle of the 128-wide diagonal strip of stx (no PE mask
    matmuls). Fully-masked columns are skipped via the narrow-diag trick.
  - ctxT[d, q] (+ sums row) = V_aug^T @ P^T via lhsT=V_aug, rhs=stx slice,
    accumulated over k tiles in PSUM (bf16 matmuls).
  - Normalize: linv = 1/sums read directly from PSUM (DVE), broadcast
    across partitions with a single SBUF->SBUF stride-0 DMA, multiply on
    DVE (PSUM operand) -> ctxn (bf16). No DRAM bounce, no PSUM->SBUF ctx
    copy.
  - outT partial [e, q] = Wo_shard^T @ ctxn per q chunk, interleaved with
    the attention of later chunks; PSUM->SBUF copies alternate DVE/Pool;
    DMA'd out as fp32.
  - Input DMAs are issued qc-chunked (xt per (qc, et) 1KB-per-partition
    chunks, weights first) so the first projection matmuls start ~3us in
    instead of waiting for the full 4MB xt fill.

Host side: gather 8 partial outT tensors, out[b] = sum_g partial^T + bo.
"""

import numpy as np
import ml_dtypes

import bass_rust
import concourse.bass as bass
import concourse.tile as tile
from concourse import mybir
from concourse.tile import TileContext
from concourse.bass_utils import run_bass_kernel_spmd
from concourse.vector_clock import ScopedClock

# ---------------------------------------------------------------------------
# Workaround: this environment's walrus build caps the number of sync-wait
# commands encodable on a single instruction. Redistribute excess waits onto
# single-wait NOPs inserted immediately before the instruction on the same
# engine queue (program order preserves the blocking semantics).
# ---------------------------------------------------------------------------


def _patched_drain_and_barrier(self, tick_clock, wait_clock):
    nop0 = self.nc.sync.nop(nofuse=True)
    wait_clock.add_sem_waits(nop0.ins, ScopedClock({None: tick_clock.global_clock}))
    si = nop0.ins.sync_info
    if si is not None and si.on_wait and len(si.on_wait) > 1:
        waits = list(si.on_wait)
        nop0.ins.sync_info = bass_rust.SyncInfo(
            on_wait=[waits[0]], on_update=list(si.on_update or [])
        )
        for w in waits[1:]:
            n = self.nc.sync.nop(nofuse=True)
            n.ins.sync_info = bass_rust.SyncInfo(on_wait=[w], on_update=[])
    self.nc.sync.drain()
    self.nc.all_engine_barrier()
    assert self.sems is not None
    popped = self.nc._tile_sem_poison_stack.pop()
    assert popped is self._sem_poison
    self.nc.clear_and_free_semaphores(list(self.sems.allocated().values()))
    self.nc.all_engine_barrier()


tile.TileContext._drain_and_barrier = _patched_drain_and_barrier

MAX_WAITS = 1


def split_excess_waits(nc, max_waits=None):
    if max_waits is None:
        max_waits = MAX_WAITS
    for f in nc.m.functions:
        for bb in f.blocks:
            insts = bb.instructions
            out = []
            changed = False
            for inst in insts:
                si = inst.sync_info
                waits = list(si.on_wait) if si is not None and si.on_wait else []
                if len(waits) > max_waits:
                    changed = True
                    excess, keep = waits[:-max_waits], waits[-max_waits:]
                    for w in excess:
                        out.append(mybir.InstNoOp(
                            name=nc.get_next_instruction_name(),
                            engine=inst.engine,
                            bass_nofuse=True,
                            sync_info=mybir.SyncInfo(on_wait=[w], on_update=[]),
                        ))
                    inst.sync_info = mybir.SyncInfo(
                        on_wait=keep, on_update=list(si.on_update or [])
                    )
                out.append(inst)
            if changed:
                bb.instructions = out


# ---------------------------------------------------------------------------
# Problem constants (hardcoded per the harness contract).
# ---------------------------------------------------------------------------

B, S, E = 2, 2048, 1024
H, D = 16, 64
NCORES, TP = 8, 4
HPC = H // TP          # heads per core = 4
DH = HPC * D           # head-dim columns per core = 256
P = 128                # partitions
QC = 512               # q chunk (free dim of score matmuls)
NQC = S // QC          # 4 q chunks
NKT = S // P           # 16 k tiles
SCALE = 1.0 / np.sqrt(np.float32(D))

OPTS = {
    "pj_bufs": 4,
    "st_bufs": 2,
    "stx_bufs": 5,
    "po_bufs": 2,
    "pv_depth": 2,
    "narrow": True,
    "ob_pool_split": False,
    "bcast": "dram",  # "pool" (gpsimd partition_broadcast) | "dram" (bounce)
}

BF = mybir.dt.bfloat16
F32 = mybir.dt.float32
FP8 = mybir.dt.float8e4
Exp = mybir.ActivationFunctionType.Exp
MULT = mybir.AluOpType.mult
DR = mybir.MatmulPerfMode.DoubleRow


class _Persist:
    pass


def _declare_io(nc):
    io = _Persist()
    # inputs are pre-arranged on host to [partition, chunk, free] so every
    # DMA is contiguous per partition
    io.xt = nc.dram_tensor("xt", [P, E // P, S], BF, kind="ExternalInput").ap()
    io.wq = nc.dram_tensor("wq", [P, E // P, DH], BF, kind="ExternalInput").ap()
    io.wk = nc.dram_tensor("wk", [P, E // P, DH], BF, kind="ExternalInput").ap()
    io.wv = nc.dram_tensor("wv", [P, E // P, DH], BF, kind="ExternalInput").ap()
    io.wo = nc.dram_tensor("wo", [P, DH // P, E], BF, kind="ExternalInput").ap()
    io.outp = nc.dram_tensor("outp", [E, S], F32, kind="ExternalOutput").ap()
    return io


ET = E // P   # 8 e tiles
NM = QC // P  # 4 k-tiles per q chunk
NDT = DH // P  # 2 head-pair column blocks


def _persistent(ctx, tc):
    nc = tc.nc
    ps = _Persist()
    consts = ctx.enter_context(tc.tile_pool(name="consts", bufs=1))

    ps.xt_t = [consts.tile([P, S], BF, tag=f"xt{et}", name=f"xt{et}")
               for et in range(ET)]
    ps.wq_sb = consts.tile([P, ET, DH], BF, tag="wq")
    ps.wk_sb = consts.tile([P, ET, DH], BF, tag="wk")
    ps.wv_sb = consts.tile([P, ET, DH], BF, tag="wv")
    ps.wo_sb = consts.tile([P, DH // P, E], BF, tag="wo")
    # fp8 Q/K with a zero second DoubleRow slot:
    # qt8[qc]: [P, pair, slot, q]; kt8[ki]: [P, pair, slot, k]
    ps.qt8_q = [consts.tile([P, NDT, 2, QC], FP8, tag=f"qt{qc}",
                            name=f"qt{qc}") for qc in range(NQC)]
    ps.kt8_t = [consts.tile([P, NDT, 2, P], FP8, tag=f"kt{ki}", name=f"kt{ki}")
                for ki in range(NKT)]
    ps.v_t = [consts.tile([P, HPC, 66], BF, tag=f"v{ki}", name=f"v{ki}")
              for ki in range(NKT)]
    ps.ctxn_q = [consts.tile([P, DH // P, QC], BF, tag=f"ctxn{qc}",
                             name=f"ctxn{qc}") for qc in range(NQC)]

    # zero DoubleRow slots (written once; iterations only touch slot 0)
    for qc in range(NQC):
        nc.gpsimd.memset(ps.qt8_q[qc][:, :, 1, :], 0.0)
    for ki in range(NKT):
        nc.gpsimd.memset(ps.kt8_t[ki][:, :, 1, :], 0.0)
        # ones column for V_aug
        nc.gpsimd.memset(ps.v_t[ki][:, :, 64:66], 0.0)
        nc.gpsimd.memset(ps.v_t[ki][:, :, 64:65], 1.0)

    ps.stx_pool = ctx.enter_context(tc.tile_pool(name="stx", bufs=OPTS["stx_bufs"]))
    ps.ctxu_pool = ctx.enter_context(tc.tile_pool(name="ctxu", bufs=4))
    ps.linv_pool = ctx.enter_context(tc.tile_pool(name="linv", bufs=8))
    ps.ob_pool = ctx.enter_context(tc.tile_pool(name="ob", bufs=4))
    if OPTS["bcast"] == "pool":
        from concourse import library_config
        nc.gpsimd.load_library(library_config.attn)
    else:
        ps.dram = ctx.enter_context(tc.tile_pool(name="dram", bufs=2, space="DRAM"))
    return ps


def _iteration(tc, io, ps):
    nc = tc.nc
    xt_t, qt8_q, kt8_t, v_t, ctxn_q = ps.xt_t, ps.qt8_q, ps.kt8_t, ps.v_t, ps.ctxn_q
    wq_sb, wk_sb, wv_sb, wo_sb = ps.wq_sb, ps.wk_sb, ps.wv_sb, ps.wo_sb

    # qc-chunked input fill: weights for Q/K first, then per-chunk xt
    # columns (1KB/partition contiguous), wv/wo slotted between chunks.
    nc.sync.dma_start(wq_sb[:], io.wq)
    nc.sync.dma_start(wk_sb[:], io.wk)
    for qc in range(NQC):
        for et in range(ET):
            nc.sync.dma_start(
                xt_t[et][:, qc * QC:(qc + 1) * QC],
                io.xt[:, et, qc * QC:(qc + 1) * QC],
            )
        if qc == 0:
            nc.sync.dma_start(wv_sb[:], io.wv)
        elif qc == 1:
            nc.sync.dma_start(wo_sb[:], io.wo)

    # ---- projections (qc-major so attention for early q chunks can start
    # while later projections are still running) ----------------------------
    with tc.tile_pool(name="pj", bufs=OPTS["pj_bufs"], space="PSUM") as pjp:
        for qc in range(NQC):
            # QT / KT for this q chunk: [d, q] layout -> fp8 slot-0 tiles
            for w_sb, is_q in ((wq_sb, True), (wk_sb, False)):
                for dt in range(NDT):
                    psum = pjp.tile([P, QC], F32, tag="pj", name="pj")
                    for et in range(ET):
                        nc.tensor.matmul(
                            psum[:],
                            lhsT=w_sb[:, et, dt * P:(dt + 1) * P],
                            rhs=xt_t[et][:, qc * QC:(qc + 1) * QC],
                            start=(et == 0), stop=(et == ET - 1),
                        )
                    if is_q:
                        nc.vector.tensor_copy(
                            out=qt8_q[qc][:, dt, 0, :], in_=psum[:]
                        )
                    else:
                        for j in range(NM):
                            nc.vector.tensor_copy(
                                out=kt8_t[qc * NM + j][:, dt, 0, :],
                                in_=psum[:, j * P:(j + 1) * P],
                            )
            # V for this chunk's k-tiles: [s, d] layout, per-head 66-wide slots
            for st in range(qc * NM, (qc + 1) * NM):
                psum = pjp.tile([P, DH], F32, tag="pjv", name="pjv")
                for et in range(ET):
                    nc.tensor.matmul(
                        psum[:],
                        lhsT=xt_t[et][:, st * P:(st + 1) * P],
                        rhs=wv_sb[:, et, :],
                        start=(et == 0), stop=(et == ET - 1),
                    )
                nc.vector.tensor_copy(
                    out=v_t[st][:, :, 0:64],
                    in_=psum.rearrange("p (h d) -> p h d", h=HPC),
                )

    # ---- attention + interleaved output projection ------------------------
    # PE is in-order, so the emission order is the schedule:
    #  - PV for unit ki is emitted `pv_depth` units late so its exp (ACT) and
    #    diag select (Pool) have two units of score-matmul time to finish.
    #  - out-proj for chunk qc-1 is laced between this chunk's units so those
    #    matmuls soak up any PE slack while ACT works on exps.
    #  - pvs PSUM banks are freed immediately after the last PV by a ctxu
    #    copy; the reciprocal/broadcast/multiply chain runs out of SBUF.
    with tc.tile_pool(name="pv", bufs=1, space="PSUM") as pvp, \
         tc.tile_pool(name="st", bufs=OPTS["st_bufs"], space="PSUM") as stp, \
         tc.tile_pool(name="po", bufs=OPTS["po_bufs"], space="PSUM") as pop:

        obn = [0]

        def emit_oproj(qcc, et, borrow=False):
            if borrow and et % 3:
                psum = pvp.tile([P, QC], F32, tag=f"pv{et % 2}", name="povv")
            else:
                psum = pop.tile([P, QC], F32, tag="po", name="po")
            for cc2 in range(DH // P):
                nc.tensor.matmul(
                    psum[:],
                    lhsT=wo_sb[:, cc2, et * P:(et + 1) * P],
                    rhs=ctxn_q[qcc][:, cc2, :],
                    start=(cc2 == 0), stop=(cc2 == DH // P - 1),
                )
            ob = ps.ob_pool.tile([P, QC], F32, tag="ob", name="ob")
            obn[0] += 1
            eng = nc.gpsimd if (OPTS["ob_pool_split"] and obn[0] % 2) else nc.vector
            eng.tensor_copy(out=ob[:], in_=psum[:])
            nc.sync.dma_start(
                io.outp.rearrange("(eo p) q -> p eo q", p=P)[
                    :, et, qcc * QC:(qcc + 1) * QC
                ],
                ob[:],
            )

        for qc in range(NQC):
            nk = (qc + 1) * NM  # causal k-tiles for this chunk
            prev_ets = list(range(ET)) if qc > 0 else []
            spread = max(1, (2 * nk) // (ET + 1)) if prev_ets else 0
            ucount = 0
            for hp in range(HPC // 2):
                cc = hp
                pvs = [pvp.tile([P, QC], F32, tag=f"pv{i}", name=f"pv{i}")
                       for i in range(2)]
                pending = []

                def emit_pv(ent):
                    ki2, stx2, off2 = ent
                    for i in range(2):
                        h = 2 * hp + i
                        nc.tensor.matmul(
                            pvs[i][0:65, off2:],
                            lhsT=v_t[ki2][:, h, 0:65],
                            rhs=stx2[:, i, off2:],
                            start=(ki2 == 0), stop=(ki2 == nk - 1),
                        )

                for ki in range(nk):
                    diag = ki >= qc * NM
                    m = ki - qc * NM if diag else 0
                    off = P * m if (diag and OPTS["narrow"]) else 0
                    # both heads' scores into one 2-bank PSUM tile via
                    # fp8 DoubleRow matmuls (slot 1 of qt8/kt8 is zero)
                    st_ps = stp.tile([P, 2, QC], F32, tag="st", name="st")
                    for i in range(2):
                        pr = 64 * i
                        nc.tensor.matmul(
                            st_ps[:, i, off:],
                            lhsT=kt8_t[ki][pr:pr + 64, cc, :, :],
                            rhs=qt8_q[qc][pr:pr + 64, cc, :, off:],
                            start=True, stop=True,
                            perf_mode=DR,
                        )
                    # one exp covers both heads (PSUM -> SBUF bf16)
                    stx = ps.stx_pool.tile([P, 2, QC], BF, tag="stx",
                                           name="stx")
                    nc.scalar.activation(
                        out=stx[:, :, off:], in_=st_ps[:, :, off:], func=Exp,
                        scale=float(SCALE),
                    )
                    if diag:
                        # zero the causally-invalid lower triangle of the
                        # diagonal 128-wide strip (Pool engine)
                        for i in range(2):
                            nc.gpsimd.affine_select(
                                out=stx[:, i, off:off + P],
                                in_=stx[:, i, off:off + P],
                                compare_op=mybir.AluOpType.is_ge, fill=0.0,
                                base=0, pattern=[[1, P]],
                                channel_multiplier=-1,
                            )
                    pending.append((ki, stx, off))
                    if len(pending) > OPTS["pv_depth"]:
                        emit_pv(pending.pop(0))
                    ucount += 1
                    if prev_ets and spread and ucount % spread == 0:
                        emit_oproj(qc - 1, prev_ets.pop(0))
                while pending:
                    emit_pv(pending.pop(0))
                # free the pv banks right away: ctx (+sums row) -> SBUF f32,
                # then normalize out of SBUF
                for i in range(2):
                    ctxu = ps.ctxu_pool.tile([65, QC], F32, tag="ctxu",
                                             name="ctxu")
                    nc.vector.tensor_copy(out=ctxu[:], in_=pvs[i][0:65, :])
                    linv = ps.linv_pool.tile([1, QC], F32, tag="linv",
                                             name="linv")
                    nc.vector.reciprocal(linv[:], ctxu[64:65, :])
                    linv_bc = ps.linv_pool.tile([64, QC], F32, tag="linvbc",
                                                name="linvbc")
                    if OPTS["bcast"] == "pool":
                        nc.gpsimd.partition_broadcast(linv_bc[:], linv[:])
                    else:
                        linv_d = ps.dram.tile([1, QC], F32, tag="linvd",
                                              name="linvd")
                        nc.sync.dma_start(linv_d[:], linv[:])
                        nc.sync.dma_start(
                            linv_bc[:], linv_d[0:1, :].to_broadcast((64, QC))
                        )
                    nc.vector.tensor_tensor(
                        ctxn_q[qc][64 * i:64 * i + 64, cc, :],
                        ctxu[0:64, :], linv_bc[:], MULT,
                    )
            while prev_ets:
                emit_oproj(qc - 1, prev_ets.pop(0))
        # last chunk's out-proj: the pv banks are free afterwards — borrow
        # them so the tail pipelines deeper than the po ring alone.
        for et in range(ET):
            emit_oproj(NQC - 1, et, borrow=True)


_NC_CACHE = {}


def build_nc(iters=1):
    if iters not in _NC_CACHE:
        from contextlib import ExitStack
        nc = bass.Bass("TRN2", target_bir_lowering=False, debug=False)
        with TileContext(nc) as tc, ExitStack() as es:
            io = _declare_io(nc)
            ps = _persistent(es, tc)
            for _ in range(iters):
                _iteration(tc, io, ps)
        split_excess_waits(nc)
        _NC_CACHE[iters] = nc
    return _NC_CACHE[iters]


def make_in_maps(embeddings, wq, wk, wv, wo):
    bf = ml_dtypes.bfloat16
    in_maps = []
    for c in range(NCORES):
        b, g = c // TP, c % TP
        cols = slice(g * DH, (g + 1) * DH)
        def _arr(a):  # [(c p), f] -> [p, c, f] contiguous
            c = a.shape[0] // 128
            return np.ascontiguousarray(
                a.reshape(c, 128, a.shape[1]).transpose(1, 0, 2)).astype(bf)
        in_maps.append({
            "xt": _arr(embeddings[b].T),
            "wq": _arr(wq[:, cols]),
            "wk": _arr(wk[:, cols]),
            "wv": _arr(wv[:, cols]),
            "wo": _arr(wo[cols, :]),
        })
    return in_maps


def assemble(results, bo):
    out = np.zeros((B, S, E), dtype=np.float32)
    for c in range(NCORES):
        b = c // TP
        out[b] += results[c]["outp"].T
    out += bo.astype(np.float32)
    return out


def kernel(embeddings, wq, wk, wv, wo, bo):
    embeddings = np.asarray(embeddings)
    nc = build_nc()
    in_maps = make_in_maps(embeddings, np.asarray(wq), np.asarray(wk),
                           np.asarray(wv), np.asarray(wo))
    res = run_bass_kernel_spmd(nc, in_maps, core_ids=list(range(NCORES)),
                               trace=False)
    return assemble(res.results, np.asarray(bo))
